# revision 7
# baseline (speedup 1.0000x reference)
"""Trainium2 Bass kernel for the soft-decision-tree ensemble classifier.

Math (per batch row b, tree t, leaf l):
  z[n]     = x[b] . W[t,n] + bias[t,n]            (63 internal nodes)
  llp[l]   = sum_{k in path(l)} dir_k*z_k - sum_k softplus(z_k)
  leafprob = exp(llp);  out[b,c] = sum_t 2*softmax(tw)_t sum_l leafprob * softmax(ll[t,l])_c

Design (v5):
  * Data-parallel over batch: B=4096 -> 512 rows/core, 8 cores, no collectives.
  * Node layout: groups of 4 trees, two 128-partition tiles per group:
      t0 = levels 0-4 (31 nodes + 1 pad per tree), t1 = level-5 nodes (32/tree).
  * Stage 1 (PE): z matmuls fp16 (raw, no bias), PSUM pairs.
  * DVE: ta = fp16(pz + bias) per tile (tensor_scalar with per-partition bias).
  * ACT: te = Exp(ta) in PAIR-sized instructions (SBUF src -> batching allowed),
         sp = Ln(te + 1) in QUAD-sized instructions.
  * Stage 2 (PE): even leaves only: pp = adirE^T ta(t0) + apathE04^T sp(t0)
    + (-I)^T sp(t1).  The dir-bias sum rides along inside ta.
  * lp_even = Exp(pp) in PAIR instructions (no bias port needed)
    lp_odd  = lp_even * te(t1) on DVE (exact sibling identity), PAIR-wide.
  * Stage 3 (PE): out[c,b] += vt^T lp over 32 lp tiles; vt = 2*softmax(tw)*
    softmax(leaf_logits) folded on host (parameter-only preprocessing).
  * Last two groups run at single-tile granularity (te from PSUM with bias
    port) to keep the drain chain short.
  * ACT table pinned once (Exp+Ln table).  DMA only on sync/gpsimd queues,
    first wt chunk is a single tile so the PE starts ASAP.
"""

import numpy as np

TREE_DEPTH = 6
T, N, D, C = 64, 63, 512, 100
L = 2**TREE_DEPTH            # 64
B = 4096
NCORES = 8
BS = B // NCORES             # 512
NG = 16                      # groups of 4 trees
NTILES = 2 * NG              # 32 node tiles (t0/t1 per group)

_NC_CACHE = {}


def _leaf_paths(depth):
    Ll = 2**depth
    idx = np.zeros((Ll, depth), np.int32)
    dr = np.zeros((Ll, depth), np.int32)
    for l in range(Ll):
        node = 0
        for k in range(depth):
            bit = (l >> (depth - 1 - k)) & 1
            idx[l, k] = node
            dr[l, k] = bit
            node = 2 * node + 1 + bit
    return idx, dr


_IDX, _DR = _leaf_paths(TREE_DEPTH)


def _host_pack(x, split_weights, split_bias, leaf_logits, tree_weights):
    """All parameter-side preprocessing (batch-independent)."""
    x = np.asarray(x, np.float32)
    W = np.asarray(split_weights, np.float32)
    b = np.asarray(split_bias, np.float32)
    ll = np.asarray(leaf_logits, np.float32)
    tw = np.asarray(tree_weights, np.float32)

    Wslot = np.zeros((NTILES, 128, D), np.float32)
    bias32 = np.zeros((128, NTILES), np.float32)
    for g in range(NG):
        for tau in range(4):
            t = 4 * g + tau
            Wslot[2 * g, 32 * tau:32 * tau + 31, :] = W[t, 0:31, :]
            bias32[32 * tau:32 * tau + 31, 2 * g] = b[t, 0:31]
            Wslot[2 * g + 1, 32 * tau:32 * tau + 32, :] = W[t, 31:63, :]
            bias32[32 * tau:32 * tau + 32, 2 * g + 1] = b[t, 31:63]
    wt16 = np.ascontiguousarray(
        Wslot.reshape(NTILES, 128, 4, 128).transpose(3, 0, 2, 1)
    ).reshape(128, NTILES * 4 * 128).astype(np.float16)

    adirE = np.zeros((128, 128), np.float32)    # [t0 slot, even-leaf]
    apathE = np.zeros((128, 128), np.float32)
    for j in range(32):
        l = 2 * j
        for k in range(5):                      # levels 0..4
            node = _IDX[l, k]
            for tau in range(4):
                if _DR[l, k]:
                    adirE[32 * tau + node, 32 * tau + j] = 1.0
                apathE[32 * tau + node, 32 * tau + j] = -1.0
    amatE = np.zeros((128, 384), np.float16)
    amatE[:, 0:128] = adirE.astype(np.float16)
    amatE[:, 128:256] = apathE.astype(np.float16)
    amatE[:, 256:384] = -np.eye(128, dtype=np.float16)

    wsm = np.exp(tw - tw.max()); wsm /= wsm.sum()
    ev = np.exp(ll - ll.max(-1, keepdims=True))
    vt = ev / ev.sum(-1, keepdims=True)
    vt = (2.0 * wsm[:, None, None] * vt).astype(np.float32)
    vt_all = np.zeros((128, NTILES, C), np.float32)
    for g in range(NG):
        for tau in range(4):
            t = 4 * g + tau
            vt_all[32 * tau:32 * tau + 32, 2 * g, :] = vt[t, 0::2, :]
            vt_all[32 * tau:32 * tau + 32, 2 * g + 1, :] = vt[t, 1::2, :]
    vt16 = np.ascontiguousarray(
        vt_all.reshape(128, NTILES * C)).astype(np.float16)

    consts = np.ascontiguousarray(bias32)

    shared = dict(wt=wt16, amat=amatE, vt=vt16, consts=consts)
    in_maps = []
    xT = np.ascontiguousarray(x.T.astype(np.float16))   # [D, B]
    for i in range(NCORES):
        xti = np.ascontiguousarray(xT[:, i * BS:(i + 1) * BS]).reshape(
            4, 128, BS).transpose(1, 0, 2).reshape(128, 4 * BS)
        in_maps.append(dict(xt=np.ascontiguousarray(xti), **shared))
    return in_maps


def _build_bass():
    import concourse.bacc as bacc
    import concourse.mybir as mybir
    import concourse.tile as tile
    from concourse.hw_specs import get_activation_tables

    dt = mybir.dt
    f32 = dt.float32
    f32r = dt.float32r
    fp16 = dt.float16
    AF = mybir.ActivationFunctionType
    ALU = mybir.AluOpType

    nc = bacc.Bacc("TRN2", target_bir_lowering=False, debug=False,
                   num_devices=NCORES)

    table_id = next(i for i, (_, funcs) in
                    enumerate(get_activation_tables("gen3").items())
                    if AF.Exp in funcs and AF.Ln in funcs)
    nc.scalar.add_instruction(mybir.InstLoadActFuncSet(
        name=f"I-{nc.next_id()}", ins=[], outs=[], act_func_set_id=table_id))

    xt = nc.dram_tensor("xt", [128, 4 * BS], fp16, kind="ExternalInput").ap()
    wt = nc.dram_tensor("wt", [128, NTILES * 4 * 128], fp16,
                        kind="ExternalInput").ap()
    amat = nc.dram_tensor("amat", [128, 384], fp16, kind="ExternalInput").ap()
    vt = nc.dram_tensor("vt", [128, NTILES * C], fp16,
                        kind="ExternalInput").ap()
    consts = nc.dram_tensor("consts", [128, NTILES], f32r,
                            kind="ExternalInput").ap()
    out = nc.dram_tensor("out", [C, BS], f32, kind="ExternalOutput").ap()

    NTAIL = 2                 # last groups run at single-tile granularity

    with tile.TileContext(nc) as tc:
        with (
            tc.tile_pool(name="big", bufs=1) as bigp,
            tc.tile_pool(name="wk2", bufs=2) as wk2,
            tc.tile_pool(name="wk3", bufs=3) as wk3,
            tc.tile_pool(name="pz", bufs=3, space="PSUM") as pzp,
            tc.tile_pool(name="pp", bufs=2, space="PSUM") as ppp,
            tc.tile_pool(name="po", bufs=1, space="PSUM") as pop,
        ):
            # ---- inputs ------------------------------------------------
            xt_t = bigp.tile([128, 4 * BS], fp16, tag="xt")
            wt_t = bigp.tile([128, NTILES * 4 * 128], fp16, tag="wt")
            amat_t = bigp.tile([128, 384], fp16, tag="amat")
            vt_t = bigp.tile([128, NTILES * C], fp16, tag="vt")
            consts_t = bigp.tile([128, NTILES], f32r, tag="consts")

            TCOL = 4 * 128           # wt columns per tile
            nc.sync.dma_start(out=xt_t[:, 0:BS], in_=xt[:, 0:BS])
            nc.gpsimd.dma_start(out=wt_t[:, 0:TCOL], in_=wt[:, 0:TCOL])
            nc.sync.dma_start(out=xt_t[:, BS:2 * BS], in_=xt[:, BS:2 * BS])
            nc.gpsimd.dma_start(out=wt_t[:, TCOL:2 * TCOL],
                                in_=wt[:, TCOL:2 * TCOL])
            nc.sync.dma_start(out=xt_t[:, 2 * BS:4 * BS],
                              in_=xt[:, 2 * BS:4 * BS])
            nc.gpsimd.dma_start(out=consts_t[:], in_=consts[:])
            nc.sync.dma_start(out=amat_t[:], in_=amat[:])
            # remaining wt in growing chunks, alternating queues
            bounds = [2, 4, 8, 12, 16, 20, 24, 28, 32]
            for ci in range(len(bounds) - 1):
                lo, hi = bounds[ci] * TCOL, bounds[ci + 1] * TCOL
                eng = nc.gpsimd if ci % 2 else nc.sync
                eng.dma_start(out=wt_t[:, lo:hi], in_=wt[:, lo:hi])
                if bounds[ci] == 12:
                    nc.sync.dma_start(out=vt_t[:, 0:NTILES * C // 2],
                                      in_=vt[:, 0:NTILES * C // 2])
                if bounds[ci] == 20:
                    nc.gpsimd.dma_start(out=vt_t[:, NTILES * C // 2:],
                                        in_=vt[:, NTILES * C // 2:])

            def bias_ap(t):
                return consts_t[:, t:t + 1].bitcast(f32)

            adir_ap = amat_t[:, 0:128]
            apath04_ap = amat_t[:, 128:256]
            negI_ap = amat_t[:, 256:384]

            out_ps = pop.tile([C, BS], f32, tag="outps")

            pend2 = []     # stage-2+lp emitters (lag 1 pair-round)
            pend3 = []     # stage-3 emitters (one more round of lag)

            def make_pair(q, ta_q, te_q, sp_q):
                """Pair-round q covers groups 2q, 2q+1 (tiles 4q..4q+3)."""
                def s2():
                    pp = ppp.tile([128, 2 * BS], f32, tag="pp",
                                  name=f"pp{q}")
                    for h, g in enumerate((2 * q, 2 * q + 1)):
                        sl = slice(h * BS, (h + 1) * BS)
                        o0, o1 = 2 * h, 2 * h + 1
                        nc.tensor.matmul(pp[:, sl], lhsT=adir_ap,
                                         rhs=ta_q[:, o0 * BS:(o0 + 1) * BS],
                                         start=True, stop=False)
                        nc.tensor.matmul(pp[:, sl], lhsT=apath04_ap,
                                         rhs=sp_q[:, o0 * BS:(o0 + 1) * BS],
                                         start=False, stop=False)
                        nc.tensor.matmul(pp[:, sl], lhsT=negI_ap,
                                         rhs=sp_q[:, o1 * BS:(o1 + 1) * BS],
                                         start=False, stop=True)
                    lpe = wk3.tile([128, 2 * BS], fp16, tag="lpe",
                                   name=f"lpe{q}")
                    nc.scalar.activation(lpe[:], pp[:], AF.Exp)
                    lpo = wk3.tile([128, 2 * BS], fp16, tag="lpo",
                                   name=f"lpo{q}")
                    nc.vector.tensor_tensor(
                        out=lpo[:].rearrange("p (a b) -> p a b", a=2),
                        in0=lpe[:].rearrange("p (a b) -> p a b", a=2),
                        in1=te_q[:].rearrange("p (a b) -> p a b", a=4)
                            [:, 1::2, :],
                        op=ALU.mult)
                    return lpe, lpo
                return s2

            def make_s3_pair(q, lpe, lpo):
                def s3():
                    for h, g in enumerate((2 * q, 2 * q + 1)):
                        sl = slice(h * BS, (h + 1) * BS)
                        nc.tensor.matmul(
                            out_ps[:],
                            lhsT=vt_t[:, (2 * g) * C:(2 * g + 1) * C],
                            rhs=lpe[:, sl], start=(g == 0), stop=False)
                        nc.tensor.matmul(
                            out_ps[:],
                            lhsT=vt_t[:, (2 * g + 1) * C:(2 * g + 2) * C],
                            rhs=lpo[:, sl], start=False,
                            stop=(g == NG - 1))
                return s3

            # ---- main pair-rounds --------------------------------------
            NQ = (NG - NTAIL) // 2
            for q in range(NQ):
                ta_q = wk2.tile([128, 4 * BS], fp16, tag="taq",
                                name=f"taq{q}")
                te_q = wk2.tile([128, 4 * BS], fp16, tag="teq",
                                name=f"teq{q}")
                sp_q = wk2.tile([128, 4 * BS], fp16, tag="spq",
                                name=f"spq{q}")
                for h in range(2):          # two groups per round
                    g = 2 * q + h
                    for hh in range(2):
                        tt = 2 * g + hh
                        pz = pzp.tile([128, BS], f32, tag="pz",
                                      name=f"pz{tt}")
                        for j in range(4):
                            col = (tt * 4 + j) * 128
                            nc.tensor.matmul(
                                pz[:],
                                lhsT=wt_t[:, col:col + 128],
                                rhs=xt_t[:, j * BS:(j + 1) * BS],
                                start=(j == 0), stop=(j == 3))
                        o = tt % 4
                        nc.vector.tensor_scalar_add(
                            out=ta_q[:, o * BS:(o + 1) * BS],
                            in0=pz[:], scalar1=bias_ap(tt))
                    oo = 2 * h
                    nc.scalar.activation(
                        te_q[:, oo * BS:(oo + 2) * BS],
                        ta_q[:, oo * BS:(oo + 2) * BS], AF.Exp)
                nc.scalar.activation(sp_q[:], te_q[:], AF.Ln,
                                     bias=1.0, scale=1.0)

                s2 = make_pair(q, ta_q, te_q, sp_q)
                s2.q = q
                pend2.append(s2)
                if len(pend2) >= 2:
                    s2d = pend2.pop(0)
                    pend3.append(make_s3_pair(s2d.q, *s2d()))
                if len(pend3) >= 2:
                    pend3.pop(0)()

            # ---- tail groups: single-tile granularity ------------------
            def tail_fwd(g):
                st = {}
                ta2 = wk3.tile([128, 2 * BS], fp16, tag="ta2",
                               name=f"ta2{g}")
                te2 = wk3.tile([128, 2 * BS], fp16, tag="te2",
                               name=f"te2{g}")
                sp2 = wk3.tile([128, 2 * BS], fp16, tag="sp2",
                               name=f"sp2{g}")
                for hh in range(2):
                    tt = 2 * g + hh
                    sl = slice(hh * BS, (hh + 1) * BS)
                    pz = pzp.tile([128, BS], f32, tag="pz", name=f"pz{tt}")
                    for j in range(4):
                        col = (tt * 4 + j) * 128
                        nc.tensor.matmul(
                            pz[:],
                            lhsT=wt_t[:, col:col + 128],
                            rhs=xt_t[:, j * BS:(j + 1) * BS],
                            start=(j == 0), stop=(j == 3))
                    nc.vector.tensor_scalar_add(out=ta2[:, sl], in0=pz[:],
                                                scalar1=bias_ap(tt))
                    nc.scalar.activation(te2[:, sl], ta2[:, sl], AF.Exp)
                    nc.scalar.activation(sp2[:, sl], te2[:, sl], AF.Ln,
                                         bias=1.0, scale=1.0)
                st['ta2'], st['te2'], st['sp2'] = ta2, te2, sp2
                return st

            def tail_bwd(g, st):
                ta2, te2, sp2 = st['ta2'], st['te2'], st['sp2']
                pp = ppp.tile([128, 2 * BS], f32, tag="pp", name=f"pp{g}t")
                nc.tensor.matmul(pp[:, 0:BS], lhsT=adir_ap, rhs=ta2[:, 0:BS],
                                 start=True, stop=False)
                nc.tensor.matmul(pp[:, 0:BS], lhsT=apath04_ap,
                                 rhs=sp2[:, 0:BS], start=False, stop=False)
                nc.tensor.matmul(pp[:, 0:BS], lhsT=negI_ap, rhs=sp2[:, BS:],
                                 start=False, stop=True)
                lpe = wk3.tile([128, BS], fp16, tag="lpe1", name=f"lpe{g}t")
                nc.scalar.activation(lpe[:], pp[:, 0:BS], AF.Exp)
                lpo = wk3.tile([128, BS], fp16, tag="lpo1", name=f"lpo{g}t")
                nc.vector.tensor_tensor(out=lpo[:], in0=lpe[:],
                                        in1=te2[:, BS:], op=ALU.mult)
                nc.tensor.matmul(out_ps[:],
                                 lhsT=vt_t[:, (2 * g) * C:(2 * g + 1) * C],
                                 rhs=lpe[:], start=(g == 0), stop=False)
                nc.tensor.matmul(
                    out_ps[:], lhsT=vt_t[:, (2 * g + 1) * C:(2 * g + 2) * C],
                    rhs=lpo[:], start=False, stop=(g == NG - 1))

            # drain pair pipeline interleaved with the tail stage-1 work
            st14 = tail_fwd(NG - 2)
            while pend2:
                s2d = pend2.pop(0)
                pend3.append(make_s3_pair(s2d.q, *s2d()))
            st15 = tail_fwd(NG - 1)
            while pend3:
                pend3.pop(0)()
            tail_bwd(NG - 2, st14)
            tail_bwd(NG - 1, st15)

            # ---- drain output (split for tail overlap) ----------------
            out_sb = bigp.tile([C, BS], f32, tag="osb")
            nc.vector.tensor_copy(out=out_sb[:, 0:BS // 2],
                                  in_=out_ps[:, 0:BS // 2])
            nc.sync.dma_start(out=out[:, 0:BS // 2],
                              in_=out_sb[:, 0:BS // 2])
            nc.vector.tensor_copy(out=out_sb[:, BS // 2:],
                                  in_=out_ps[:, BS // 2:])
            nc.sync.dma_start(out=out[:, BS // 2:],
                              in_=out_sb[:, BS // 2:])

    nc.finalize()
    return nc


def _get_nc():
    if "nc" not in _NC_CACHE:
        _NC_CACHE["nc"] = _build_bass()
    return _NC_CACHE["nc"]


def _prep_inputs(x, split_weights, split_bias, leaf_logits, tree_weights):
    return _host_pack(x, split_weights, split_bias, leaf_logits, tree_weights)


def kernel(x, split_weights, split_bias, leaf_logits, tree_weights):
    from concourse.bass_utils import run_bass_kernel_spmd

    in_maps = _prep_inputs(x, split_weights, split_bias, leaf_logits,
                           tree_weights)
    nc = _get_nc()
    res = run_bass_kernel_spmd(nc, in_maps, core_ids=list(range(NCORES)))
    out = np.concatenate([res.results[i]["out"] for i in range(NCORES)],
                         axis=1).T                              # [B, C]
    return np.ascontiguousarray(out.astype(np.float32))


# revision 10
# speedup vs baseline: 1.0034x; 1.0034x over previous
"""Trainium2 Bass kernel for the soft-decision-tree ensemble classifier.

Math (per batch row b, tree t, leaf l):
  z[n]     = x[b] . W[t,n] + bias[t,n]            (63 internal nodes)
  llp[l]   = sum_{k in path(l)} dir_k*z_k - sum_k softplus(z_k)
  leafprob = exp(llp);  out[b,c] = sum_t 2*softmax(tw)_t sum_l leafprob * softmax(ll[t,l])_c

Design (v6):
  * Data-parallel over batch: B=4096 -> 512 rows/core, 8 cores, no collectives.
  * Node layout: groups of 4 trees, two 128-partition tiles per group:
      t0 = levels 0-4 (31 nodes + 1 pad per tree), t1 = level-5 nodes (32/tree).
  * Stage 1 (PE): z matmuls fp16 (raw, no bias) into per-group PSUM pairs.
  * DVE: ta = fp16(pz + bias) pair-wide tensor_tensor with stride-0 bias
    broadcast (one instruction per group).
  * ACT: te = Exp(ta) pair-wide; sp = Ln(te + 1) quad-wide.
  * Stage 2 (PE): even leaves only: pp = adirE^T ta(t0) + apathE04^T sp(t0)
    + (-I)^T sp(t1); dir-bias rides inside ta.
  * lp_even = Exp(pp) (ACT, per group); lp_odd = lp_even * te(t1) (DVE) -
    exact sibling identity, so only half the leaf exponentials are computed.
  * Stage 3 (PE): out[c,b] += vt^T lp over 32 lp tiles; vt = 2*softmax(tw)*
    softmax(leaf_logits) folded on host (parameter-only preprocessing).
  * ~10us of dummy matmuls on uninitialized SBUF warm the PE p-state during
    the input-DMA window (the PE clock only reaches full speed after ~3us of
    continuous busy).
  * All DMA on the sync queue (gpsimd/scalar queues left untouched - DMA on
    them inflates their exit drains / blocks ACT).  First wt chunk is a
    single tile so real work starts ASAP.
  * Deferred stage-2/3 emission (lag 2/3 groups) keeps the PE stream dense;
    the last two groups run fine-grained for a short drain chain.
"""

import numpy as np

TREE_DEPTH = 6
T, N, D, C = 64, 63, 512, 100
L = 2**TREE_DEPTH            # 64
B = 4096
NCORES = 8
BS = B // NCORES             # 512
NG = 16                      # groups of 4 trees
NTILES = 2 * NG              # 32 node tiles (t0/t1 per group)

_NC_CACHE = {}


def _leaf_paths(depth):
    Ll = 2**depth
    idx = np.zeros((Ll, depth), np.int32)
    dr = np.zeros((Ll, depth), np.int32)
    for l in range(Ll):
        node = 0
        for k in range(depth):
            bit = (l >> (depth - 1 - k)) & 1
            idx[l, k] = node
            dr[l, k] = bit
            node = 2 * node + 1 + bit
    return idx, dr


_IDX, _DR = _leaf_paths(TREE_DEPTH)


def _host_pack(x, split_weights, split_bias, leaf_logits, tree_weights):
    """All parameter-side preprocessing (batch-independent)."""
    x = np.asarray(x, np.float32)
    W = np.asarray(split_weights, np.float32)
    b = np.asarray(split_bias, np.float32)
    ll = np.asarray(leaf_logits, np.float32)
    tw = np.asarray(tree_weights, np.float32)

    Wslot = np.zeros((NTILES, 128, D), np.float32)
    bias32 = np.zeros((128, NTILES), np.float32)
    for g in range(NG):
        for tau in range(4):
            t = 4 * g + tau
            Wslot[2 * g, 32 * tau:32 * tau + 31, :] = W[t, 0:31, :]
            bias32[32 * tau:32 * tau + 31, 2 * g] = b[t, 0:31]
            Wslot[2 * g + 1, 32 * tau:32 * tau + 32, :] = W[t, 31:63, :]
            bias32[32 * tau:32 * tau + 32, 2 * g + 1] = b[t, 31:63]
    wt16 = np.ascontiguousarray(
        Wslot.reshape(NTILES, 128, 4, 128).transpose(3, 0, 2, 1)
    ).reshape(128, NTILES * 4 * 128).astype(np.float16)

    adirE = np.zeros((128, 128), np.float32)    # [t0 slot, even-leaf]
    apathE = np.zeros((128, 128), np.float32)
    for j in range(32):
        l = 2 * j
        for k in range(5):                      # levels 0..4
            node = _IDX[l, k]
            for tau in range(4):
                if _DR[l, k]:
                    adirE[32 * tau + node, 32 * tau + j] = 1.0
                apathE[32 * tau + node, 32 * tau + j] = -1.0
    amatE = np.zeros((128, 384), np.float16)
    amatE[:, 0:128] = adirE.astype(np.float16)
    amatE[:, 128:256] = apathE.astype(np.float16)
    amatE[:, 256:384] = -np.eye(128, dtype=np.float16)

    wsm = np.exp(tw - tw.max()); wsm /= wsm.sum()
    ev = np.exp(ll - ll.max(-1, keepdims=True))
    vt = ev / ev.sum(-1, keepdims=True)
    vt = (2.0 * wsm[:, None, None] * vt).astype(np.float32)
    vt_all = np.zeros((128, NTILES, C), np.float32)
    for g in range(NG):
        for tau in range(4):
            t = 4 * g + tau
            vt_all[32 * tau:32 * tau + 32, 2 * g, :] = vt[t, 0::2, :]
            vt_all[32 * tau:32 * tau + 32, 2 * g + 1, :] = vt[t, 1::2, :]
    vt16 = np.ascontiguousarray(
        vt_all.reshape(128, NTILES * C)).astype(np.float16)

    consts = np.ascontiguousarray(bias32)

    shared = dict(wt=wt16, amat=amatE, vt=vt16, consts=consts)
    in_maps = []
    xT = np.ascontiguousarray(x.T.astype(np.float16))   # [D, B]
    for i in range(NCORES):
        xti = np.ascontiguousarray(xT[:, i * BS:(i + 1) * BS]).reshape(
            4, 128, BS).transpose(1, 0, 2).reshape(128, 4 * BS)
        in_maps.append(dict(xt=np.ascontiguousarray(xti), **shared))
    return in_maps


def _build_bass():
    import concourse.bacc as bacc
    import concourse.mybir as mybir
    import concourse.tile as tile
    from concourse.hw_specs import get_activation_tables

    dt = mybir.dt
    f32 = dt.float32
    f32r = dt.float32r
    fp16 = dt.float16
    AF = mybir.ActivationFunctionType
    ALU = mybir.AluOpType

    nc = bacc.Bacc("TRN2", target_bir_lowering=False, debug=False,
                   num_devices=NCORES)

    table_id = next(i for i, (_, funcs) in
                    enumerate(get_activation_tables("gen3").items())
                    if AF.Exp in funcs and AF.Ln in funcs)
    nc.scalar.add_instruction(mybir.InstLoadActFuncSet(
        name=f"I-{nc.next_id()}", ins=[], outs=[], act_func_set_id=table_id))

    xt = nc.dram_tensor("xt", [128, 4 * BS], fp16, kind="ExternalInput").ap()
    wt = nc.dram_tensor("wt", [128, NTILES * 4 * 128], fp16,
                        kind="ExternalInput").ap()
    amat = nc.dram_tensor("amat", [128, 384], fp16, kind="ExternalInput").ap()
    vt = nc.dram_tensor("vt", [128, NTILES * C], fp16,
                        kind="ExternalInput").ap()
    consts = nc.dram_tensor("consts", [128, NTILES], f32r,
                            kind="ExternalInput").ap()
    out = nc.dram_tensor("out", [C, BS], f32, kind="ExternalOutput").ap()

    with tile.TileContext(nc) as tc:
        with (
            tc.tile_pool(name="big", bufs=1) as bigp,
            tc.tile_pool(name="wk2", bufs=2) as wk2,
            tc.tile_pool(name="wk4", bufs=4) as wk4,
            tc.tile_pool(name="pz", bufs=2, space="PSUM") as pzp,
            tc.tile_pool(name="pp", bufs=3, space="PSUM") as ppp,
            tc.tile_pool(name="po", bufs=1, space="PSUM") as pop,
        ):
            # ---- inputs (sync queue only) -----------------------------
            xt_t = bigp.tile([128, 4 * BS], fp16, tag="xt")
            wt_t = bigp.tile([128, NTILES * 4 * 128], fp16, tag="wt")
            amat_t = bigp.tile([128, 384], fp16, tag="amat")
            vt_t = bigp.tile([128, NTILES * C], fp16, tag="vt")
            consts_t = bigp.tile([128, NTILES], f32r, tag="consts")

            TCOL = 4 * 128
            nc.sync.dma_start(out=xt_t[:, 0:BS], in_=xt[:, 0:BS])
            nc.sync.dma_start(out=wt_t[:, 0:TCOL], in_=wt[:, 0:TCOL])
            nc.sync.dma_start(out=xt_t[:, BS:4 * BS], in_=xt[:, BS:4 * BS])
            nc.sync.dma_start(out=consts_t[:], in_=consts[:])
            nc.sync.dma_start(out=amat_t[:], in_=amat[:])
            bounds = [1, 2, 4, 8, 12, 16, 22, 27, 32]
            for ci in range(len(bounds) - 1):
                lo, hi = bounds[ci] * TCOL, bounds[ci + 1] * TCOL
                nc.sync.dma_start(out=wt_t[:, lo:hi], in_=wt[:, lo:hi])
                if bounds[ci] == 12:
                    nc.sync.dma_start(out=vt_t[:, 0:NTILES * C // 2],
                                      in_=vt[:, 0:NTILES * C // 2])
                if bounds[ci] == 22:
                    nc.sync.dma_start(out=vt_t[:, NTILES * C // 2:],
                                      in_=vt[:, NTILES * C // 2:])

            adir_ap = amat_t[:, 0:128]
            apath04_ap = amat_t[:, 128:256]
            negI_ap = amat_t[:, 256:384]

            out_ps = pop.tile([C, BS], f32, tag="outps")

            # ---- PE p-state warmup ------------------------------------
            junk = bigp.tile([128, 640], fp16, tag="junk")
            nc.gpsimd.memset(junk[:], 0.0)
            pwarm = ppp.tile([128, BS], f32, tag="pp", name="warm")
            for _ in range(48):
                nc.tensor.matmul(pwarm[:], lhsT=junk[:, 0:128],
                                 rhs=junk[:, 128:640], start=True, stop=True)

            pend2 = []
            pend3 = []

            def make_s2(g, ta_t, te_t, sp_q):
                def s2():
                    pp = ppp.tile([128, BS], f32, tag="pp", name=f"pp{g}")
                    o0 = (2 * g) % 4
                    o1 = o0 + 1
                    nc.tensor.matmul(pp[:], lhsT=adir_ap, rhs=ta_t[:, 0:BS],
                                     start=True, stop=False)
                    nc.tensor.matmul(pp[:], lhsT=apath04_ap,
                                     rhs=sp_q[:, o0 * BS:(o0 + 1) * BS],
                                     start=False, stop=False)
                    nc.tensor.matmul(pp[:], lhsT=negI_ap,
                                     rhs=sp_q[:, o1 * BS:(o1 + 1) * BS],
                                     start=False, stop=True)
                    lpe = wk4.tile([128, BS], fp16, tag="lpe",
                                   name=f"lpe{g}")
                    nc.scalar.activation(lpe[:], pp[:], AF.Exp)
                    lpo = wk4.tile([128, BS], fp16, tag="lpo",
                                   name=f"lpo{g}")
                    nc.vector.tensor_tensor(out=lpo[:], in0=lpe[:],
                                            in1=te_t[:, BS:2 * BS],
                                            op=ALU.mult)
                    return lpe, lpo
                return s2

            def make_s3(g, lpe, lpo):
                def s3():
                    nc.tensor.matmul(
                        out_ps[:], lhsT=vt_t[:, (2 * g) * C:(2 * g + 1) * C],
                        rhs=lpe[:], start=(g == 0), stop=False)
                    nc.tensor.matmul(
                        out_ps[:],
                        lhsT=vt_t[:, (2 * g + 1) * C:(2 * g + 2) * C],
                        rhs=lpo[:], start=False, stop=(g == NG - 1))
                return s3

            def emit_group(g, fine_tail):
                """Stage 1 + bias-add + te/sp for group g."""
                pz = pzp.tile([128, 2 * BS], f32, tag="pz", name=f"pz{g}")
                for hh in range(2):
                    tt = 2 * g + hh
                    for j in range(4):
                        col = (tt * 4 + j) * 128
                        nc.tensor.matmul(
                            pz[:, hh * BS:(hh + 1) * BS],
                            lhsT=wt_t[:, col:col + 128],
                            rhs=xt_t[:, j * BS:(j + 1) * BS],
                            start=(j == 0), stop=(j == 3))
                ta_t = wk4.tile([128, 2 * BS], fp16, tag="ta", name=f"ta{g}")
                nc.vector.tensor_tensor(
                    out=ta_t[:].rearrange("p (a b) -> p a b", a=2),
                    in0=pz[:].rearrange("p (a b) -> p a b", a=2),
                    in1=consts_t[:, 2 * g:2 * g + 2].bitcast(f32)
                        .unsqueeze(2).broadcast_to([128, 2, BS]),
                    op=ALU.add)
                te_t = wk4.tile([128, 2 * BS], fp16, tag="te", name=f"te{g}")
                if fine_tail:
                    nc.scalar.activation(te_t[:, 0:BS], ta_t[:, 0:BS],
                                         AF.Exp)
                    nc.scalar.activation(te_t[:, BS:], ta_t[:, BS:], AF.Exp)
                else:
                    nc.scalar.activation(te_t[:], ta_t[:], AF.Exp)
                return pz, ta_t, te_t

            sp_q = None
            for g in range(NG - 2):
                if g % 2 == 0:
                    sp_q = wk2.tile([128, 4 * BS], fp16, tag="spq",
                                    name=f"spq{g//2}")
                pz, ta_t, te_t = emit_group(g, False)
                # per-pair Ln (half-quad): batched but low-latency
                nc.scalar.activation(sp_q[:, (2 * g % 4) * BS:
                                          ((2 * g % 4) + 2) * BS],
                                     te_t[:], AF.Ln, bias=1.0, scale=1.0)

                s2 = make_s2(g, ta_t, te_t, sp_q)
                s2.g = g
                pend2.append(s2)
                if len(pend2) >= 3:
                    s2d = pend2.pop(0)
                    pend3.append(make_s3(s2d.g, *s2d()))
                if len(pend3) >= 2:
                    pend3.pop(0)()

            # ---- tail: last two groups, fine grained -------------------
            tails = {}
            for g in (NG - 2, NG - 1):
                sp_t = wk2.tile([128, 2 * BS], fp16, tag="spt",
                                name=f"spt{g}")
                pz, ta_t, te_t = emit_group(g, True)
                nc.scalar.activation(sp_t[:, 0:BS], te_t[:, 0:BS], AF.Ln,
                                     bias=1.0, scale=1.0)
                nc.scalar.activation(sp_t[:, BS:], te_t[:, BS:], AF.Ln,
                                     bias=1.0, scale=1.0)
                tails[g] = (ta_t, te_t, sp_t)
                # drain one pending pair while the tail ACT chain runs
                while pend2:
                    s2d = pend2.pop(0)
                    pend3.append(make_s3(s2d.g, *s2d()))
                if g == NG - 2:
                    while len(pend3) > 1:
                        pend3.pop(0)()

            while pend3:
                pend3.pop(0)()

            for g in (NG - 2, NG - 1):
                ta_t, te_t, sp_t = tails[g]
                pp = ppp.tile([128, BS], f32, tag="pp", name=f"pp{g}")
                nc.tensor.matmul(pp[:], lhsT=adir_ap, rhs=ta_t[:, 0:BS],
                                 start=True, stop=False)
                nc.tensor.matmul(pp[:], lhsT=apath04_ap, rhs=sp_t[:, 0:BS],
                                 start=False, stop=False)
                nc.tensor.matmul(pp[:], lhsT=negI_ap, rhs=sp_t[:, BS:],
                                 start=False, stop=True)
                lpe = wk4.tile([128, BS], fp16, tag="lpe", name=f"lpe{g}")
                nc.scalar.activation(lpe[:], pp[:], AF.Exp)
                lpo = wk4.tile([128, BS], fp16, tag="lpo", name=f"lpo{g}")
                nc.vector.tensor_tensor(out=lpo[:], in0=lpe[:],
                                        in1=te_t[:, BS:], op=ALU.mult)
                make_s3(g, lpe, lpo)()

            # ---- drain output -----------------------------------------
            out_sb = bigp.tile([C, BS], f32, tag="osb")
            nc.vector.tensor_copy(out=out_sb[:, 0:BS // 2],
                                  in_=out_ps[:, 0:BS // 2])
            nc.sync.dma_start(out=out[:, 0:BS // 2],
                              in_=out_sb[:, 0:BS // 2])
            nc.vector.tensor_copy(out=out_sb[:, BS // 2:],
                                  in_=out_ps[:, BS // 2:])
            nc.sync.dma_start(out=out[:, BS // 2:],
                              in_=out_sb[:, BS // 2:])

    nc.finalize()
    return nc


def _get_nc():
    if "nc" not in _NC_CACHE:
        _NC_CACHE["nc"] = _build_bass()
    return _NC_CACHE["nc"]


def _prep_inputs(x, split_weights, split_bias, leaf_logits, tree_weights):
    return _host_pack(x, split_weights, split_bias, leaf_logits, tree_weights)


def kernel(x, split_weights, split_bias, leaf_logits, tree_weights):
    from concourse.bass_utils import run_bass_kernel_spmd

    in_maps = _prep_inputs(x, split_weights, split_bias, leaf_logits,
                           tree_weights)
    nc = _get_nc()
    res = run_bass_kernel_spmd(nc, in_maps, core_ids=list(range(NCORES)))
    out = np.concatenate([res.results[i]["out"] for i in range(NCORES)],
                         axis=1).T                              # [B, C]
    return np.ascontiguousarray(out.astype(np.float32))


# revision 17
# speedup vs baseline: 1.0859x; 1.0822x over previous
"""Trainium2 Bass kernel for the soft-decision-tree ensemble classifier.

Math (per batch row b, tree t, leaf l):
  z[n]     = x[b] . W[t,n] + bias[t,n]            (63 internal nodes)
  llp[l]   = sum_{k in path(l)} dir_k*z_k - sum_k softplus(z_k)
  leafprob = exp(llp);  out[b,c] = sum_t 2*softmax(tw)_t sum_l leafprob * softmax(ll[t,l])_c

Design (v6):
  * Data-parallel over batch: B=4096 -> 512 rows/core, 8 cores, no collectives.
  * Node layout: groups of 4 trees, two 128-partition tiles per group:
      t0 = levels 0-4 (31 nodes + 1 pad per tree), t1 = level-5 nodes (32/tree).
  * Stage 1 (PE): z matmuls fp16 (raw, no bias) into per-group PSUM pairs.
  * DVE: ta = fp16(pz + bias) pair-wide tensor_tensor with stride-0 bias
    broadcast (one instruction per group).
  * ACT: te = Exp(ta) pair-wide; sp = Ln(te + 1) quad-wide.
  * Stage 2 (PE): even leaves only: pp = adirE^T ta(t0) + apathE04^T sp(t0)
    + (-I)^T sp(t1); dir-bias rides inside ta.
  * lp_even = Exp(pp) (ACT, per group); lp_odd = lp_even * te(t1) (DVE) -
    exact sibling identity, so only half the leaf exponentials are computed.
  * Stage 3 (PE): out[c,b] += vt^T lp over 32 lp tiles; vt = 2*softmax(tw)*
    softmax(leaf_logits) folded on host (parameter-only preprocessing).
  * ~10us of dummy matmuls on uninitialized SBUF warm the PE p-state during
    the input-DMA window (the PE clock only reaches full speed after ~3us of
    continuous busy).
  * All DMA on the sync queue (gpsimd/scalar queues left untouched - DMA on
    them inflates their exit drains / blocks ACT).  First wt chunk is a
    single tile so real work starts ASAP.
  * Deferred stage-2/3 emission (lag 2/3 groups) keeps the PE stream dense;
    the last two groups run fine-grained for a short drain chain.
"""

import numpy as np

TREE_DEPTH = 6
T, N, D, C = 64, 63, 512, 100
L = 2**TREE_DEPTH            # 64
B = 4096
NCORES = 8
BS = B // NCORES             # 512
NG = 16                      # groups of 4 trees
NTILES = 2 * NG              # 32 node tiles (t0/t1 per group)

_NC_CACHE = {}


def _leaf_paths(depth):
    Ll = 2**depth
    idx = np.zeros((Ll, depth), np.int32)
    dr = np.zeros((Ll, depth), np.int32)
    for l in range(Ll):
        node = 0
        for k in range(depth):
            bit = (l >> (depth - 1 - k)) & 1
            idx[l, k] = node
            dr[l, k] = bit
            node = 2 * node + 1 + bit
    return idx, dr


_IDX, _DR = _leaf_paths(TREE_DEPTH)


def _host_pack(x, split_weights, split_bias, leaf_logits, tree_weights):
    """All parameter-side preprocessing (batch-independent)."""
    x = np.asarray(x, np.float32)
    W = np.asarray(split_weights, np.float32)
    b = np.asarray(split_bias, np.float32)
    ll = np.asarray(leaf_logits, np.float32)
    tw = np.asarray(tree_weights, np.float32)

    Wslot = np.zeros((NTILES, 128, D), np.float32)
    bias32 = np.zeros((128, NTILES), np.float32)
    for g in range(NG):
        for tau in range(4):
            t = 4 * g + tau
            Wslot[2 * g, 32 * tau:32 * tau + 31, :] = W[t, 0:31, :]
            bias32[32 * tau:32 * tau + 31, 2 * g] = b[t, 0:31]
            Wslot[2 * g + 1, 32 * tau:32 * tau + 32, :] = W[t, 31:63, :]
            bias32[32 * tau:32 * tau + 32, 2 * g + 1] = b[t, 31:63]
    wt16 = np.ascontiguousarray(
        Wslot.reshape(NTILES, 128, 4, 128).transpose(3, 0, 2, 1)
    ).reshape(128, NTILES * 4 * 128).astype(np.float16)

    adirE = np.zeros((128, 128), np.float32)    # [t0 slot, even-leaf]
    apathE = np.zeros((128, 128), np.float32)
    for j in range(32):
        l = 2 * j
        for k in range(5):                      # levels 0..4
            node = _IDX[l, k]
            for tau in range(4):
                if _DR[l, k]:
                    adirE[32 * tau + node, 32 * tau + j] = 1.0
                apathE[32 * tau + node, 32 * tau + j] = -1.0
    amatE = np.zeros((128, 384), np.float16)
    amatE[:, 0:128] = adirE.astype(np.float16)
    amatE[:, 128:256] = apathE.astype(np.float16)
    amatE[:, 256:384] = -np.eye(128, dtype=np.float16)

    wsm = np.exp(tw - tw.max()); wsm /= wsm.sum()
    ev = np.exp(ll - ll.max(-1, keepdims=True))
    vt = ev / ev.sum(-1, keepdims=True)
    vt = (2.0 * wsm[:, None, None] * vt).astype(np.float32)
    vt_all = np.zeros((128, NTILES, C), np.float32)
    for g in range(NG):
        for tau in range(4):
            t = 4 * g + tau
            vt_all[32 * tau:32 * tau + 32, 2 * g, :] = vt[t, 0::2, :]
            vt_all[32 * tau:32 * tau + 32, 2 * g + 1, :] = vt[t, 1::2, :]
    vt16 = np.ascontiguousarray(
        vt_all.reshape(128, NTILES * C)).astype(np.float16)

    # aux: [amatE 384 | bias16 32 | vt 3200] all fp16, one DMA
    aux = np.zeros((128, 384 + NTILES + NTILES * C), np.float16)
    aux[:, 0:384] = amatE
    aux[:, 384:384 + NTILES] = bias32.astype(np.float16)
    aux[:, 384 + NTILES:] = vt16

    shared = dict(wt=wt16, aux=np.ascontiguousarray(aux))
    in_maps = []
    xT = np.ascontiguousarray(x.T.astype(np.float16))   # [D, B]
    for i in range(NCORES):
        xti = np.ascontiguousarray(xT[:, i * BS:(i + 1) * BS]).reshape(
            4, 128, BS).transpose(1, 0, 2).reshape(128, 4 * BS)
        in_maps.append(dict(xt=np.ascontiguousarray(xti), **shared))
    return in_maps


def _build_bass():
    import concourse.bacc as bacc
    import concourse.mybir as mybir
    import concourse.tile as tile
    from concourse.hw_specs import get_activation_tables

    dt = mybir.dt
    f32 = dt.float32
    f32r = dt.float32r
    fp16 = dt.float16
    AF = mybir.ActivationFunctionType
    ALU = mybir.AluOpType

    nc = bacc.Bacc("TRN2", target_bir_lowering=False, debug=False,
                   num_devices=NCORES)

    table_id = next(i for i, (_, funcs) in
                    enumerate(get_activation_tables("gen3").items())
                    if AF.Exp in funcs and AF.Ln in funcs)
    nc.scalar.add_instruction(mybir.InstLoadActFuncSet(
        name=f"I-{nc.next_id()}", ins=[], outs=[], act_func_set_id=table_id))

    AUXW = 384 + NTILES + NTILES * C
    xt = nc.dram_tensor("xt", [128, 4 * BS], fp16, kind="ExternalInput").ap()
    wt = nc.dram_tensor("wt", [128, NTILES * 4 * 128], fp16,
                        kind="ExternalInput").ap()
    aux = nc.dram_tensor("aux", [128, AUXW], fp16,
                         kind="ExternalInput").ap()
    out = nc.dram_tensor("out", [C, BS], f32, kind="ExternalOutput").ap()

    with tile.TileContext(nc) as tc:
        with (
            tc.tile_pool(name="big", bufs=1) as bigp,
            tc.tile_pool(name="wk2", bufs=3) as wk2,
            tc.tile_pool(name="wk4", bufs=4) as wk4,
            tc.tile_pool(name="pz", bufs=2, space="PSUM") as pzp,
            tc.tile_pool(name="pp", bufs=3, space="PSUM") as ppp,
            tc.tile_pool(name="po", bufs=1, space="PSUM") as pop,
        ):
            # ---- inputs (sync queue only) -----------------------------
            xt_t = bigp.tile([128, 4 * BS], fp16, tag="xt")
            wt_t = bigp.tile([128, NTILES * 4 * 128], fp16, tag="wt")
            aux_t = bigp.tile([128, AUXW], fp16, tag="aux")

            TCOL = 4 * 128
            nc.sync.dma_start(out=xt_t[:, 0:BS], in_=xt[:, 0:BS])
            nc.sync.dma_start(out=wt_t[:, 0:TCOL], in_=wt[:, 0:TCOL])
            nc.sync.dma_start(out=xt_t[:, BS:4 * BS], in_=xt[:, BS:4 * BS])
            nc.sync.dma_start(out=aux_t[:], in_=aux[:])
            bounds = [1, 4, 10, 16, 24, 32]
            for ci in range(len(bounds) - 1):
                lo, hi = bounds[ci] * TCOL, bounds[ci + 1] * TCOL
                nc.sync.dma_start(out=wt_t[:, lo:hi], in_=wt[:, lo:hi])

            adir_ap = aux_t[:, 0:128]
            apath04_ap = aux_t[:, 128:256]
            negI_ap = aux_t[:, 256:384]
            vt_t = aux_t[:, 384 + NTILES:]

            out_ps = pop.tile([C, BS], f32, tag="outps")

            # ---- PE p-state warmup ------------------------------------
            junk = bigp.tile([128, 640], fp16, tag="junk")
            nc.vector.memset(junk[:], 0.0)
            pwarm = ppp.tile([128, BS], f32, tag="pp", name="warm")
            for _ in range(18):
                nc.tensor.matmul(pwarm[:], lhsT=junk[:, 0:128],
                                 rhs=junk[:, 128:640], start=True, stop=True)

            pend2 = []
            pend3 = []

            def make_s2(g, ta_t, te_ap, sp_q):
                def s2():
                    pp = ppp.tile([128, BS], f32, tag="pp", name=f"pp{g}")
                    o0 = (2 * g) % 4
                    o1 = o0 + 1
                    nc.tensor.matmul(pp[:], lhsT=adir_ap, rhs=ta_t[:, 0:BS],
                                     start=True, stop=False)
                    nc.tensor.matmul(pp[:], lhsT=apath04_ap,
                                     rhs=sp_q[:, o0 * BS:(o0 + 1) * BS],
                                     start=False, stop=False)
                    nc.tensor.matmul(pp[:], lhsT=negI_ap,
                                     rhs=sp_q[:, o1 * BS:(o1 + 1) * BS],
                                     start=False, stop=True)
                    lpe = wk4.tile([128, BS], fp16, tag="lpe",
                                   name=f"lpe{g}")
                    nc.scalar.activation(lpe[:], pp[:], AF.Exp)
                    lpo = wk4.tile([128, BS], fp16, tag="lpo",
                                   name=f"lpo{g}")
                    nc.vector.tensor_tensor(out=lpo[:], in0=lpe[:],
                                            in1=te_ap,
                                            op=ALU.mult)
                    return lpe, lpo
                return s2

            def make_s3(g, lpe, lpo):
                def s3():
                    nc.tensor.matmul(
                        out_ps[:], lhsT=vt_t[:, (2 * g) * C:(2 * g + 1) * C],
                        rhs=lpe[:], start=(g == 0), stop=False)
                    nc.tensor.matmul(
                        out_ps[:],
                        lhsT=vt_t[:, (2 * g + 1) * C:(2 * g + 2) * C],
                        rhs=lpo[:], start=False, stop=(g == NG - 1))
                return s3

            def emit_s1_bias(g, ta_t):
                """Stage-1 matmuls + pair-wide bias add into ta_t."""
                pz = pzp.tile([128, 2 * BS], f32, tag="pz", name=f"pz{g}")
                for hh in range(2):
                    tt = 2 * g + hh
                    for j in range(4):
                        col = (tt * 4 + j) * 128
                        nc.tensor.matmul(
                            pz[:, hh * BS:(hh + 1) * BS],
                            lhsT=wt_t[:, col:col + 128],
                            rhs=xt_t[:, j * BS:(j + 1) * BS],
                            start=(j == 0), stop=(j == 3))
                nc.vector.tensor_tensor(
                    out=ta_t[:].rearrange("p (a b) -> p a b", a=2),
                    in0=pz[:].rearrange("p (a b) -> p a b", a=2),
                    in1=aux_t[:, 384 + 2 * g:384 + 2 * g + 2]
                        .unsqueeze(2).broadcast_to([128, 2, BS]),
                    op=ALU.add)

            te_q = sp_q = None
            for g in range(NG - 2):
                if g % 2 == 0:
                    te_q = wk2.tile([128, 4 * BS], fp16, tag="teq",
                                    name=f"teq{g//2}")
                    sp_q = wk2.tile([128, 4 * BS], fp16, tag="spq",
                                    name=f"spq{g//2}")
                ta_t = wk4.tile([128, 2 * BS], fp16, tag="ta", name=f"ta{g}")
                emit_s1_bias(g, ta_t)
                o0 = (2 * g) % 4
                nc.scalar.activation(te_q[:, o0 * BS:(o0 + 2) * BS],
                                     ta_t[:], AF.Exp)
                if g % 2 == 1:
                    nc.scalar.activation(sp_q[:], te_q[:], AF.Ln,
                                         bias=1.0, scale=1.0)

                s2 = make_s2(g, ta_t,
                             te_q[:, (o0 + 1) * BS:(o0 + 2) * BS], sp_q)
                s2.g = g
                pend2.append(s2)
                if len(pend2) >= 3:
                    s2d = pend2.pop(0)
                    pend3.append(make_s3(s2d.g, *s2d()))
                if len(pend3) >= 2:
                    pend3.pop(0)()

            # ---- tail: last two groups, fine grained -------------------
            tails = {}
            for g in (NG - 2, NG - 1):
                ta_t = wk4.tile([128, 2 * BS], fp16, tag="ta", name=f"ta{g}")
                te_t = wk4.tile([128, 2 * BS], fp16, tag="te", name=f"te{g}")
                sp_t = wk2.tile([128, 2 * BS], fp16, tag="spt",
                                name=f"spt{g}")
                emit_s1_bias(g, ta_t)
                for hh in range(2):
                    sl = slice(hh * BS, (hh + 1) * BS)
                    nc.scalar.activation(te_t[:, sl], ta_t[:, sl], AF.Exp)
                    nc.scalar.activation(sp_t[:, sl], te_t[:, sl], AF.Ln,
                                         bias=1.0, scale=1.0)
                tails[g] = (ta_t, te_t, sp_t)
                # drain pending pipeline while the tail ACT chain runs
                while pend2:
                    s2d = pend2.pop(0)
                    pend3.append(make_s3(s2d.g, *s2d()))
                if g == NG - 2:
                    while len(pend3) > 1:
                        pend3.pop(0)()

            while pend3:
                pend3.pop(0)()

            for g in (NG - 2, NG - 1):
                ta_t, te_t, sp_t = tails[g]
                pp = ppp.tile([128, BS], f32, tag="pp", name=f"pp{g}")
                nc.tensor.matmul(pp[:], lhsT=adir_ap, rhs=ta_t[:, 0:BS],
                                 start=True, stop=False)
                nc.tensor.matmul(pp[:], lhsT=apath04_ap, rhs=sp_t[:, 0:BS],
                                 start=False, stop=False)
                nc.tensor.matmul(pp[:], lhsT=negI_ap, rhs=sp_t[:, BS:],
                                 start=False, stop=True)
                lpe = wk4.tile([128, BS], fp16, tag="lpe", name=f"lpe{g}")
                nc.scalar.activation(lpe[:], pp[:], AF.Exp)
                lpo = wk4.tile([128, BS], fp16, tag="lpo", name=f"lpo{g}")
                nc.vector.tensor_tensor(out=lpo[:], in0=lpe[:],
                                        in1=te_t[:, BS:], op=ALU.mult)
                make_s3(g, lpe, lpo)()

            # ---- drain output -----------------------------------------
            out_sb = bigp.tile([C, BS], f32, tag="osb")
            nc.vector.tensor_copy(out=out_sb[:, 0:BS // 2],
                                  in_=out_ps[:, 0:BS // 2])
            nc.sync.dma_start(out=out[:, 0:BS // 2],
                              in_=out_sb[:, 0:BS // 2])
            nc.vector.tensor_copy(out=out_sb[:, BS // 2:],
                                  in_=out_ps[:, BS // 2:])
            nc.sync.dma_start(out=out[:, BS // 2:],
                              in_=out_sb[:, BS // 2:])

    nc.finalize()
    return nc


def _get_nc():
    if "nc" not in _NC_CACHE:
        _NC_CACHE["nc"] = _build_bass()
    return _NC_CACHE["nc"]


def _prep_inputs(x, split_weights, split_bias, leaf_logits, tree_weights):
    return _host_pack(x, split_weights, split_bias, leaf_logits, tree_weights)


def kernel(x, split_weights, split_bias, leaf_logits, tree_weights):
    from concourse.bass_utils import run_bass_kernel_spmd

    in_maps = _prep_inputs(x, split_weights, split_bias, leaf_logits,
                           tree_weights)
    nc = _get_nc()
    res = run_bass_kernel_spmd(nc, in_maps, core_ids=list(range(NCORES)))
    out = np.concatenate([res.results[i]["out"] for i in range(NCORES)],
                         axis=1).T                              # [B, C]
    return np.ascontiguousarray(out.astype(np.float32))
